# revision 1
# baseline (speedup 1.0000x reference)
"""CrossAttentionBlock TRN2 kernel.

Full inputs -> shard batch dim over 8 NeuronCores (data parallel, 4 batches
each) -> Bass/Tile kernel per core -> gather outputs.

Shapes (hardcoded): x [32,512,32,32] f32, t [32,77,768] f32,
Wq [512,512], Wkv [1024,768], Wp [512,512]; out [32,512,32,32].

Per-core plan (B_local=4, C=512, HW=1024, L=77, D=768, heads=8, hd=64):
  GroupNorm(32 groups): channel-on-partition layout [128,1024] x4 tiles;
    bn_stats per channel, group aggregation + per-channel expansion via tiny
    PE matmuls with constant selection matrices (host-provided).
    rsqrt computed as exp(-0.5*ln(var+eps)) so the ACT engine never switches
    activation-table sets (ln/exp/copy live in one set).
  q = WqT.T @ xn        (f32r matmuls, N=512)
  LayerNorm(t): [77,768] row layout, then PE-transpose to [768,77].
  kv = t_lnT.T @ WkvT   -> [77, 1024] (l on partitions)
  per head h: k = kv[:, 128h:128h+64] transposed to [64,77] (PE),
    v stays [77(s), 64(c)] = natural lhsT for the AV matmul.
    attnT[s,t] = k.T @ q_head     ([77,512] x2, f32r)
    exp = Exp(0.125 * attnT)      (ACT, psum->sbuf; max-sub skipped, fp32 safe)
    hU[c,t]   = v.T @ exp         (f32r)  -- unnormalized
    den[c,t]  = ones.T @ exp      (f32r)  -- softmax denom broadcast to 64 rows
    h = hU * recip_approx(den)    (DVE)
  out = WpT.T @ h + bp + x        (f32r matmuls; bias on DVE, residual GPSIMD)

Software pipeline (keeps the PE dense so the HAM clock gate stays at 2.4GHz):
the attention phase of batch b has, interleaved between its heads, the
out-projection chunks of batch b-1 and the GroupNorm-stats/xn/q-projection of
batch b+1.
"""

import os
import sys

import numpy as np

for _p in ("/opt/trn_rl_repo", "/root/.axon_site/_ro/trn_rl_repo"):
    if _p not in sys.path and os.path.isdir(_p):
        sys.path.append(_p)

import concourse.bass as bass
import concourse.tile as tile
from concourse import bacc, mybir
from concourse.bass_utils import run_bass_kernel_spmd

F32 = mybir.dt.float32
F32R = mybir.dt.float32r
EPS = 1e-5

N_CORES = 8
B, C, H, W = 32, 512, 32, 32
HW = H * W
L, D = 77, 768
NH, HD = 8, 64
NG, GS = 32, 16  # groups, channels per group
BL = B // N_CORES  # local batches per core

LAST_RESULTS = None
_CACHE = {}


def _dma_split(nc, out_ap, in_ap, n, nsplit, eng=None):
    step = n // nsplit
    for j in range(nsplit):
        sl = slice(step * j, step * (j + 1))
        (eng or nc.sync).dma_start(out=out_ap[:, sl], in_=in_ap[:, sl])


def _build_program():
    nc = bacc.Bacc("TRN2", target_bir_lowering=False, debug=False)

    x_l = nc.declare_dram_parameter("x_l", [BL, C, HW], F32, isOutput=False)
    t_l = nc.declare_dram_parameter("t_l", [BL, L, D], F32, isOutput=False)
    wqt = nc.declare_dram_parameter("wqt", [C, C], F32R, isOutput=False)
    wkvt = nc.declare_dram_parameter("wkvt", [D, 2 * C], F32R, isOutput=False)
    wpt = nc.declare_dram_parameter("wpt", [C, C], F32R, isOutput=False)
    cblk = nc.declare_dram_parameter("cblk", [128, 148], F32, isOutput=False)
    gselt = nc.declare_dram_parameter("gselt", [8, 128], F32, isOutput=False)
    lnwb = nc.declare_dram_parameter("lnwb", [1, 2 * D], F32, isOutput=False)
    ones64 = nc.declare_dram_parameter("ones64", [128, 64], F32R, isOutput=False)
    out_l = nc.declare_dram_parameter("out_l", [BL, C, HW], F32, isOutput=True)

    TT = mybir.AluOpType
    AF = mybir.ActivationFunctionType

    from contextlib import ExitStack

    with tile.TileContext(nc) as tc, ExitStack() as ctx:
        ep = ctx.enter_context
        consts = ep(tc.tile_pool(name="consts", bufs=1))
        xp = ep(tc.tile_pool(name="xp", bufs=10))
        qpool = ep(tc.tile_pool(name="qp", bufs=5))
        hpool = ep(tc.tile_pool(name="hp", bufs=8))
        opool = ep(tc.tile_pool(name="op", bufs=4))
        tpool = ep(tc.tile_pool(name="tp", bufs=2))
        kvpool = ep(tc.tile_pool(name="kvp", bufs=2))
        ktpool = ep(tc.tile_pool(name="ktp", bufs=8))
        ttpool = ep(tc.tile_pool(name="ttp", bufs=1))
        expool = ep(tc.tile_pool(name="exp", bufs=3))
        rcpool = ep(tc.tile_pool(name="rcp", bufs=2))
        rxpool = ep(tc.tile_pool(name="rxp", bufs=6))
        spool = ep(tc.tile_pool(name="sp", bufs=4))
        abpool = ep(tc.tile_pool(name="abp", bufs=6))
        psmm = ep(tc.tile_pool(name="psmm", bufs=6, space="PSUM"))
        pstr = ep(tc.tile_pool(name="pstr", bufs=2, space="PSUM"))

        # ---- batch-0 inputs first (the sync DMA ring completes in order,
        # ~2.5us per dma_start), then merged constants, then weights ----
        pre_x = {}
        pre_t = {}

        def _pre_load(b0, nsplit=1):
            xbts = []
            for i in range(4):
                xt = xp.tile([128, HW], F32, tag="x", name="xt")
                _dma_split(
                    nc, xt, x_l[b0, 128 * i : 128 * (i + 1), :], HW, nsplit
                )
                xbts.append(xt)
            pre_x[b0] = xbts
            tbt = tpool.tile([L, D], F32, tag="t", name="tbt")
            _dma_split(nc, tbt, t_l[b0, :, :], D, 1)
            pre_t[b0] = tbt

        _pre_load(0)

        # cblk: [128, 148] = gsel(8) | ident(128) | gnw(4) | gnb(4) | bp(4)
        cblk_sb = consts.tile([128, 148], F32, tag="cblk")
        nc.sync.dma_start(out=cblk_sb, in_=cblk[:, :])
        gsel_sb = cblk_sb[:, 0:8]
        ident_sb = cblk_sb[:, 8:136]
        gnw_sb = cblk_sb[:, 136:140]
        gnb_sb = cblk_sb[:, 140:144]
        bp_sb = cblk_sb[:, 144:148]
        gselt_sb = consts.tile([8, 128], F32, tag="gselt")
        nc.sync.dma_start(out=gselt_sb, in_=gselt[:, :])
        ones_sb = consts.tile([128, 64], F32R, tag="ones64")
        nc.sync.dma_start(out=ones_sb, in_=ones64[:, :])
        # lnwb broadcast: [1, 1536] -> [128, 1536]; lnw | lnb
        lnwb_sb = consts.tile([128, 2 * D], F32, tag="lnwb")
        nc.sync.dma_start(out=lnwb_sb, in_=lnwb[:, :].to_broadcast([128, 2 * D]))
        lnw_sb = lnwb_sb[:, 0:D]
        lnb_sb = lnwb_sb[:, D : 2 * D]
        eps_sb = consts.tile([128, 1], F32, tag="eps")
        nc.vector.memset(eps_sb, EPS)

        wqt_sb = []
        for ki in range(4):
            tq = consts.tile([128, C], F32R, tag=f"wqt{ki}")
            _dma_split(nc, tq, wqt[128 * ki : 128 * (ki + 1), :], C, 1)
            wqt_sb.append(tq)
        _pre_load(1, 4)
        wkvt_sb = []
        for di in range(6):
            tk = consts.tile([128, 2 * C], F32R, tag=f"wkvt{di}")
            _dma_split(nc, tk, wkvt[128 * di : 128 * (di + 1), :], 2 * C, 1, nc.gpsimd)
            wkvt_sb.append(tk)
        wpt_sb = []
        for ki in range(4):
            tp_ = consts.tile([128, C], F32R, tag=f"wpt{ki}")
            _dma_split(nc, tp_, wpt[128 * ki : 128 * (ki + 1), :], C, 1, nc.gpsimd)
            wpt_sb.append(tp_)

        # ---------------- phase emitters ----------------
        xb_map = dict(pre_x)
        ab_map = {}
        xn_map = {}
        q_map = {}
        kv_map = {}
        tT_map = {}
        kT_map = {}
        hs_map = {}

        def x_load(b):
            if b in xb_map or b >= BL:
                return
            xbts = []
            for i in range(4):
                xt = xp.tile([128, HW], F32, tag="x")
                _dma_split(nc, xt, x_l[b, 128 * i : 128 * (i + 1), :], HW, 1)
                xbts.append(xt)
            xb_map[b] = xbts

        def x_stats(b):
            """bn_stats per channel -> per-group A'(rs), C'(-mu*rs) -> per
            channel ab tiles."""
            xb = xb_map[b]
            mv = spool.tile([128, 4, 2], F32, tag="mv")
            for i in range(4):
                st = spool.tile([128, 4, 6], F32, tag="bnst")
                for j in range(4):
                    nc.vector.bn_stats(
                        out=st[:, j, :], in_=xb[i][:, 256 * j : 256 * (j + 1)]
                    )
                nc.vector.bn_aggr(out=mv[:, i, :], in_=st)
            me2 = spool.tile([128, 4, 2], F32, tag="me2")
            nc.vector.tensor_copy(out=me2[:, :, 0], in_=mv[:, :, 0])
            nc.vector.tensor_tensor(
                out=me2[:, :, 1], in0=mv[:, :, 0], in1=mv[:, :, 0], op=TT.mult
            )
            nc.vector.tensor_tensor(
                out=me2[:, :, 1], in0=me2[:, :, 1], in1=mv[:, :, 1], op=TT.add
            )
            gpsum = pstr.tile([8, 8], F32, tag="tr")
            nc.tensor.matmul(
                out=gpsum,
                lhsT=gsel_sb,
                rhs=me2.rearrange("p a b -> p (a b)"),
                start=True,
                stop=True,
            )
            # gsel is pre-scaled by 1/16 on the host, so gpsum holds the
            # group mean and group E[x^2] directly
            gp_v = gpsum.rearrange("p (a b) -> p a b", b=2)
            gsq = spool.tile([8, 4], F32, tag="gsq")
            nc.scalar.activation(
                out=gsq, in_=gp_v[:, :, 0], func=AF.Square
            )
            gvar = spool.tile([8, 4], F32, tag="gvar")
            nc.vector.tensor_tensor(
                out=gvar, in0=gp_v[:, :, 1], in1=gsq, op=TT.subtract
            )
            # rs = 1/sqrt(var+eps) = exp(-0.5*ln(var+eps)); ln+exp share one
            # ACT table set, so no ACT_TABLE_LOAD stalls.  gac[:, :, 0]=rs,
            # gac[:, :, 1]=mu*rs (subtracted via op1 in the xn tensor_scalar)
            gln = spool.tile([8, 4], F32, tag="gln")
            nc.scalar.activation(
                out=gln, in_=gvar, func=AF.Ln, bias=eps_sb[0:8, :]
            )
            gac = spool.tile([8, 4, 2], F32, tag="gac")
            nc.scalar.activation(
                out=gac[:, :, 0], in_=gln, func=AF.Exp, scale=-0.5
            )
            nc.vector.tensor_tensor(
                out=gac[:, :, 1], in0=gp_v[:, :, 0], in1=gac[:, :, 0],
                op=TT.mult,
            )
            abs_ = []
            for i in range(4):
                epsum = pstr.tile([128, 2], F32, tag="tr")
                nc.tensor.matmul(
                    out=epsum,
                    lhsT=gselt_sb[0:8, :],
                    rhs=gac[:, i, :],
                    start=True,
                    stop=True,
                )
                ab = abpool.tile([128, 2], F32, tag="ab")
                nc.vector.tensor_tensor(
                    out=ab[:, 0:1],
                    in0=epsum[:, 0:1],
                    in1=gnw_sb[:, i : i + 1],
                    op=TT.mult,
                )
                nc.vector.tensor_tensor(
                    out=ab[:, 1:2],
                    in0=epsum[:, 1:2],
                    in1=gnw_sb[:, i : i + 1],
                    op=TT.mult,
                )
                nc.vector.tensor_tensor(
                    out=ab[:, 1:2],
                    in0=ab[:, 1:2],
                    in1=gnb_sb[:, i : i + 1],
                    op=TT.subtract,
                )
                abs_.append(ab)
            ab_map[b] = abs_

        def x_xn(b):
            xb = xb_map.pop(b)
            abs_ = ab_map.pop(b)
            xn = []
            for i in range(4):
                xnt = xp.tile([128, HW], F32R, tag="x", name="xnt")
                nc.vector.tensor_scalar(
                    out=xnt,
                    in0=xb[i],
                    scalar1=abs_[i][:, 0:1],
                    scalar2=abs_[i][:, 1:2],
                    op0=TT.mult,
                    op1=TT.subtract,
                )
                xn.append(xnt)
            xn_map[b] = xn

        def x_q(b, mi):
            xn = xn_map[b]
            qt = qpool.tile([128, HW], F32R, tag="q")
            for nh in range(2):
                qps = psmm.tile([128, 512], F32, tag="mm")
                for ki in range(4):
                    nc.tensor.matmul(
                        out=qps,
                        lhsT=wqt_sb[ki][:, 128 * mi : 128 * (mi + 1)],
                        rhs=xn[ki][:, 512 * nh : 512 * (nh + 1)],
                        start=(ki == 0),
                        stop=(ki == 3),
                    )
                nc.scalar.copy(out=qt[:, 512 * nh : 512 * (nh + 1)], in_=qps)
            q_map.setdefault(b, []).append(qt)
            if mi == 3:
                xn_map.pop(b)

        def t_ln(b):
            """LayerNorm + kv projection + per-head k transposes."""
            if b in pre_t:
                tb = pre_t[b]
            else:
                tb = tpool.tile([L, D], F32, tag="t")
                _dma_split(nc, tb, t_l[b, :, :], D, 1)
            stt = spool.tile([L, 3, 6], F32, tag="stt")
            for j in range(3):
                nc.vector.bn_stats(
                    out=stt[:, j, :], in_=tb[:, 256 * j : 256 * (j + 1)]
                )
            mvt = spool.tile([L, 2], F32, tag="mvt")
            nc.vector.bn_aggr(out=mvt, in_=stt)
            lnt = spool.tile([L, 1], F32, tag="lnt")
            nc.scalar.activation(
                out=lnt, in_=mvt[:, 1:2], func=AF.Ln, bias=eps_sb[0:L, :]
            )
            rst = spool.tile([L, 1], F32, tag="rst")
            nc.scalar.activation(out=rst, in_=lnt, func=AF.Exp, scale=-0.5)
            tn = tpool.tile([L, D], F32, tag="tn")
            nc.vector.tensor_scalar(
                out=tn,
                in0=tb,
                scalar1=mvt[:, 0:1],
                scalar2=rst,
                op0=TT.subtract,
                op1=TT.mult,
            )
            nc.vector.tensor_tensor(out=tn, in0=tn, in1=lnw_sb[0:L, :], op=TT.mult)
            nc.vector.tensor_tensor(out=tn, in0=tn, in1=lnb_sb[0:L, :], op=TT.add)

            tT = ttpool.tile([128, 6, L], F32R, tag="tT")
            for di in range(6):
                tps = pstr.tile([128, L], F32, tag="tr")
                nc.tensor.transpose(
                    tps, tn[:, 128 * di : 128 * (di + 1)], ident_sb[0:L, 0:L]
                )
                nc.scalar.copy(out=tT[:, di, :], in_=tps)

            tT_map[b] = tT

        def t_kv(b):
            tT = tT_map.pop(b)
            kv = kvpool.tile([L, 2 * C], F32R, tag="kv")
            for nh in range(2):
                kvps = psmm.tile([128, 512], F32, tag="mm")
                for di in range(6):
                    nc.tensor.matmul(
                        out=kvps[0:L, :],
                        lhsT=tT[:, di, :],
                        rhs=wkvt_sb[di][:, 512 * nh : 512 * (nh + 1)],
                        start=(di == 0),
                        stop=(di == 5),
                    )
                nc.scalar.copy(
                    out=kv[:, 512 * nh : 512 * (nh + 1)], in_=kvps[0:L, :]
                )
            kT = []
            for hp in range(4):
                kT.append(ktpool.tile([128, L], F32R, tag="kT", name="kT"))
            for h in range(NH):
                ktps = pstr.tile([128, L], F32, tag="tr")
                nc.tensor.transpose(
                    ktps[0:HD, :],
                    kv[:, 128 * h : 128 * h + HD].bitcast(F32),
                    ident_sb[0:L, 0:L],
                )
                nc.scalar.copy(
                    out=kT[h // 2][64 * (h % 2) : 64 * (h % 2) + 64, :],
                    in_=ktps[0:HD, :],
                )
            kv_map[b] = kv
            kT_map[b] = kT

        def attn_half(b, h, nh):
            q = q_map[b]
            kv = kv_map[b]
            kT = kT_map[b]
            hsb = hs_map[b]
            hp, hh = h // 2, h % 2
            sl = slice(512 * nh, 512 * (nh + 1))
            ex = expool.tile([L, 512], F32R, tag="ex", name="ex")
            atps = psmm.tile([128, 512], F32, tag="mm")
            nc.tensor.matmul(
                out=atps[0:L, :],
                lhsT=kT[hp][64 * hh : 64 * hh + 64, :],
                rhs=q[hp][64 * hh : 64 * hh + 64, sl],
                start=True,
                stop=True,
            )
            nc.scalar.activation(
                out=ex, in_=atps[0:L, :], func=AF.Exp, scale=0.125
            )
            rc = rcpool.tile([64, 512], F32, tag="rc", name="rc")
            hups = psmm.tile([128, 512], F32, tag="mm")
            nc.tensor.matmul(
                out=hups[0:HD, :], lhsT=kv[:, 128 * h + HD : 128 * (h + 1)],
                rhs=ex, start=True, stop=True,
            )
            dbps = psmm.tile([128, 512], F32, tag="mm")
            nc.tensor.matmul(
                out=dbps[0:HD, :], lhsT=ones_sb[0:L, :], rhs=ex,
                start=True, stop=True,
            )
            nc.vector.reciprocal_approx_fast(out=rc, in_=dbps[0:HD, :])
            nc.vector.tensor_tensor(
                out=hsb[hp][64 * hh : 64 * hh + 64, sl],
                in0=hups[0:HD, :],
                in1=rc,
                op=TT.mult,
            )

        rx_map = {}

        def rx_load(b):
            rxs = []
            for mi in range(4):
                rx = rxpool.tile([128, HW], F32, tag="rx", name="rx")
                _dma_split(nc, rx, x_l[b, 128 * mi : 128 * (mi + 1), :], HW, 1)
                rxs.append(rx)
            rx_map[b] = rxs

        def out_half(b, mi, nh):
            hsb = hs_map[b]
            rx = rx_map[b][mi]
            sl = slice(512 * nh, 512 * (nh + 1))
            ob = opool.tile([128, 512], F32, tag="o", name="ob")
            ops = psmm.tile([128, 512], F32, tag="mm")
            for ki in range(4):
                nc.tensor.matmul(
                    out=ops,
                    lhsT=wpt_sb[ki][:, 128 * mi : 128 * (mi + 1)],
                    rhs=hsb[ki][:, sl],
                    start=(ki == 0),
                    stop=(ki == 3),
                )
            nc.scalar.activation(
                out=ob, in_=ops, func=AF.Identity,
                bias=bp_sb[:, mi : mi + 1],
            )
            nc.gpsimd.tensor_tensor(
                out=ob, in0=ob, in1=rx[:, sl], op=TT.add
            )
            nc.sync.dma_start(
                out=out_l[b, 128 * mi : 128 * (mi + 1), sl], in_=ob
            )

        def out_done(b):
            rx_map.pop(b)
            hs_map.pop(b)

        def out_chunk(b, mi):
            out_half(b, mi, 0)
            out_half(b, mi, 1)
            if mi == 3:
                out_done(b)

        # ---------------- pipelined emission ----------------
        x_stats(0)
        x_xn(0)
        for mi in range(4):
            x_q(0, mi)
        t_ln(0)
        t_kv(0)

        for b in range(BL):
            hs_map[b] = [hpool.tile([128, HW], F32R, tag="h", name="hsb")
                         for _ in range(4)]
            if b >= 1:
                rx_load(b - 1)
            for h in range(NH):
                attn_half(b, h, 0)
                attn_half(b, h, 1)
                if b + 1 < BL:
                    if h == 1:
                        x_stats(b + 1)
                    elif h == 2:
                        t_ln(b + 1)
                    elif h == 3:
                        x_xn(b + 1)
                    elif h == 4:
                        t_kv(b + 1)
                    elif h == 5:
                        x_q(b + 1, 0)
                        x_q(b + 1, 1)
                    elif h == 6:
                        x_load(b + 2)
                    elif h == 7:
                        x_q(b + 1, 2)
                        x_q(b + 1, 3)
                if b >= 1 and h % 2 == 1:
                    out_chunk(b - 1, h // 2)
            q_map.pop(b, None)

        rx_load(BL - 1)
        for mi in range(4):
            out_chunk(BL - 1, mi)

    nc.compile()
    return nc


def _host_constants(inputs):
    f = np.float32
    wqt = np.ascontiguousarray(np.asarray(inputs["Wq"], f).T)
    wkvt = np.ascontiguousarray(np.asarray(inputs["Wkv"], f).T)
    wpt = np.ascontiguousarray(np.asarray(inputs["Wp"], f).T)
    gnw4 = np.asarray(inputs["gn_w"], f).reshape(4, 128).T
    gnb4 = np.asarray(inputs["gn_b"], f).reshape(4, 128).T
    bp4 = np.asarray(inputs["bp"], f).reshape(4, 128).T
    gsel = np.kron(np.eye(8, dtype=f), np.ones((16, 1), f))
    gselt = np.ascontiguousarray(gsel.T)
    gsel = gsel / np.float32(16.0)
    ident = np.eye(128, dtype=f)
    cblk = np.ascontiguousarray(
        np.concatenate([gsel, ident, gnw4, gnb4, bp4], axis=1)
    )
    lnwb = np.concatenate(
        [np.asarray(inputs["ln_w"], f), np.asarray(inputs["ln_b"], f)]
    ).reshape(1, 2 * D)
    ones64 = np.ones((128, 64), f)
    return dict(
        wqt=wqt, wkvt=wkvt, wpt=wpt, cblk=cblk, gselt=gselt, lnwb=lnwb,
        ones64=ones64,
    )


def kernel(**inputs):
    global LAST_RESULTS
    if "nc" not in _CACHE:
        _CACHE["nc"] = _build_program()
    nc = _CACHE["nc"]

    consts = _host_constants(inputs)
    x = np.asarray(inputs["x"], np.float32).reshape(B, C, HW)
    t = np.asarray(inputs["t"], np.float32)

    in_maps = []
    for c in range(N_CORES):
        m = dict(consts)
        m["x_l"] = np.ascontiguousarray(x[BL * c : BL * (c + 1)])
        m["t_l"] = np.ascontiguousarray(t[BL * c : BL * (c + 1)])
        in_maps.append(m)

    res = run_bass_kernel_spmd(nc, in_maps, list(range(N_CORES)))
    LAST_RESULTS = res
    out = np.concatenate([res.results[c]["out_l"] for c in range(N_CORES)], axis=0)
    return out.reshape(B, C, H, W)

